# revision 17
# baseline (speedup 1.0000x reference)
"""CrossNet layer kernel for Trainium2 (8 NeuronCores, data parallel).

Computes: out = X * (X @ alphas)[:, None] + bias + X
        = X * (1 + X @ alphas)[:, None] + bias

X: [16384, 4096] f32, alphas: [4096] f32, bias: [4096] f32.

Sharding: X split along batch into 8 row-shards of [2048, 4096]; alphas/bias
replicated (tiny, loaded once per core and broadcast across partitions
on-chip so no replicated DRAM traffic).

The kernel is pure DMA-bound (no data reuse: each X element is read once,
each out element written once), so HBM bytes are the whole cost. The host
quantizes X to bf16 before upload and upcasts the bf16 result after --
device HBM traffic is 32 MiB/core instead of 64 MiB, a 2x win. The row dot
product accumulates in f32, so the only error sources are the bf16
roundings of X / alphas / products / out (~2.7e-3 L2 rel err vs 2e-2 gate).

Measured DMA ceiling: the two HWDGE rings together sustain ~427 GB/s
(SBUF-fabric limit, 16 AXI ports x 32 B x 850 MHz), so the 33.6 MiB
stream needs ~79 us; compute must stay off that critical path.

Per [128, 4104] bf16 tile (x padded with [1,0x7] columns so the reduce
accumulates 1 + x.a directly, folding the "+ X" residual):
  1. DVE tensor_tensor:  o2 = x (*) a      (2x bf16 rate, 2.3 us)
  2. reduce: s1 = sum_free(o2) -- ACT activation(Copy, accum_out) 3.7 us
     for 14 tiles, DVE tensor_reduce 4.4 us for 2 tiles (queue balance:
     both engines ~66 us < 79 us DMA)
  3. DVE tensor_scalar:  o = x * s1        (4x bf16 rate, 1.2 us)
  4. store on the ACT HWDGE ring right behind its producer.
Emission is software-pipelined (reduce 1 tile behind the multiply,
scale/store 2 behind) so cross-engine hops never stall a queue; all 16
x loads are issued upfront into dedicated buffers so the SP ring streams
back-to-back; alphas (8 KiB) load first on the SP ring, then GpSimd
broadcasts them to 128 partitions under the first x load.
"""

import os
import sys

for _p in ("/opt/trn_rl_repo",):
    if _p not in sys.path and os.path.isdir(_p):
        sys.path.insert(0, _p)

import ml_dtypes
import numpy as np

import concourse.bacc as bacc
import concourse.bass as bass
import concourse.mybir as mybir
from concourse.bass_utils import run_bass_kernel_spmd
from concourse.tile import TileContext

N_CORES = 8
B_FULL = 16384
D = 4096
# X and alphas are padded host-side with 8 extra columns [1,0,0,0,0,0,0,0]:
# the fused multiply-reduce over 4104 elements then accumulates
# 1 + x.a directly (the pad products are 1*1 + 1*0*7), folding the "+ X"
# residual term into the TSP pass with no extra DVE op.
DP = D + 8
R = B_FULL // N_CORES  # rows per core
P = 128  # partitions

BF16 = ml_dtypes.bfloat16

_CACHE = {}


def _build(has_bias: bool) -> bass.Bass:
    f32 = mybir.dt.float32
    bf16 = mybir.dt.bfloat16
    nc = bacc.Bacc("TRN2", target_bir_lowering=False)
    x = nc.dram_tensor("x", (R, DP), bf16, kind="ExternalInput")
    a0 = nc.dram_tensor("a0", (1, DP), bf16, kind="ExternalInput")
    if has_bias:
        b0 = nc.dram_tensor("b0", (1, D), bf16, kind="ExternalInput")
    out = nc.dram_tensor("out", (R, D), bf16, kind="ExternalOutput")

    n_tiles = R // P
    mult = mybir.AluOpType.mult
    add = mybir.AluOpType.add
    bypass = mybir.AluOpType.bypass

    # Per tile: DVE tensor_tensor multiply at the 2x bf16 rate (2.25 us)
    # and DVE tensor_scalar scale at the 4x rate (1.16 us); the free-dim
    # reduce of the products runs on ACT via activation-accum (3.9 us) for
    # most tiles, on DVE tensor_reduce (4.33 us) for DVE_REDUCE tiles to
    # balance the queues (DVE ~63 us, ACT ~65 us total). Emission is
    # software-pipelined (reduce one tile behind the multiply, scale/store
    # two behind) so no engine ever waits on a same-tile cross-engine hop.
    dve_reduce = {0, 8}

    with TileContext(nc) as tc:
        with tc.tile_pool(name="const", bufs=1) as cpool:
            # alphas: 8 KiB row, first on the SP ring (delays x0 by ~0.2 us
            # only), then GpSimd broadcast to all partitions while x0/x1
            # stream in. a_t is ready ~13 us, right as x0 lands.
            a0_t = cpool.tile([1, DP], bf16)
            nc.sync.dma_start(out=a0_t, in_=a0[:, :])
            a_t = cpool.tile([P, DP], bf16)
            nc.gpsimd.partition_broadcast(a_t, a0_t)
            if has_bias:
                b0_t = cpool.tile([1, D], bf16)
                nc.scalar.dma_start(out=b0_t, in_=b0[:, :])
                b_t = cpool.tile([P, D], bf16)
                nc.gpsimd.partition_broadcast(b_t, b0_t)
            with tc.tile_pool(name="work", bufs=3) as pool:
                # Every x tile gets its own buffer (16 x ~8 KiB/partition):
                # all 16 loads are issued upfront with zero reuse-waits, so
                # the SP ring streams continuously at HBM rate and the
                # compute pipeline never starves.
                x_tiles = []
                for i in range(n_tiles):
                    t = pool.tile([P, DP], bf16, tag="x", bufs=n_tiles)
                    nc.sync.dma_start(out=t, in_=x[i * P : (i + 1) * P, :])
                    x_tiles.append(t)

                o2_tiles = {}
                s1_tiles = {}

                def stage_mul(i):
                    # o2 = x (*) a elementwise, bf16 products (2x DVE rate)
                    o2_t = pool.tile([P, DP], bf16, tag="o2", bufs=3)
                    nc.vector.tensor_tensor(
                        out=o2_t, in0=x_tiles[i], in1=a_t, op=mult
                    )
                    o2_tiles[i] = o2_t

                def stage_reduce(i):
                    # s1 = sum_free(o2) = 1 + x.a (f32 accum; the +1 comes
                    # from the host-side pad columns)
                    s1_t = pool.tile([P, 1], f32, tag="s1", bufs=4)
                    o2_t = o2_tiles[i]
                    if i in dve_reduce:
                        nc.vector.tensor_reduce(
                            out=s1_t,
                            in_=o2_t,
                            axis=mybir.AxisListType.X,
                            op=add,
                        )
                    else:
                        # ACT: dummy in-place copy, accumulator = row sum
                        nc.scalar.activation(
                            out=o2_t,
                            in_=o2_t,
                            func=mybir.ActivationFunctionType.Copy,
                            accum_out=s1_t,
                        )
                    s1_tiles[i] = s1_t

                def stage_scale_store(i):
                    o_t = pool.tile([P, D], bf16, tag="o", bufs=4)
                    if has_bias:
                        nc.vector.scalar_tensor_tensor(
                            out=o_t,
                            in0=x_tiles[i][:, 0:D],
                            scalar=s1_tiles.pop(i),
                            in1=b_t,
                            op0=mult,
                            op1=add,
                        )
                    else:
                        # DVE tensor_scalar: o = x * s1 (4x bf16 rate)
                        nc.vector.tensor_scalar_mul(
                            o_t, x_tiles[i][:, 0:D], s1_tiles.pop(i)
                        )
                    o2_tiles.pop(i)
                    nc.scalar.dma_start(
                        out=out[i * P : (i + 1) * P, :], in_=o_t
                    )

                for i in range(n_tiles):
                    stage_mul(i)
                    if i >= 1:
                        stage_reduce(i - 1)
                    if i >= 2:
                        stage_scale_store(i - 2)
                stage_reduce(n_tiles - 1)
                stage_scale_store(n_tiles - 2)
                stage_scale_store(n_tiles - 1)
    nc.compile()
    return nc


def _run(X, alphas, bias, trace=False, trace_kwargs=None):
    X = np.asarray(X)
    alphas = np.asarray(alphas)
    bias = np.asarray(bias)
    assert X.shape == (B_FULL, D), X.shape

    # Pad columns [1,0,0,0,0,0,0,0] so the on-device multiply-reduce
    # accumulates 1 + x.a directly. alphas are pre-replicated to all 128
    # partitions host-side (1 MiB once) to skip the on-device broadcast.
    pad = np.zeros((1, DP - D), dtype=BF16)
    pad[0, 0] = 1.0
    Xb = np.empty((B_FULL, DP), dtype=BF16)
    Xb[:, :D] = X.astype(BF16)
    Xb[:, D:] = pad
    ab = np.empty((1, DP), dtype=BF16)
    ab[0, :D] = alphas.astype(BF16)
    ab[:, D:] = pad

    has_bias = bool(np.any(bias))
    if has_bias not in _CACHE:
        _CACHE[has_bias] = _build(has_bias)
    nc = _CACHE[has_bias]

    in_maps = []
    for c in range(N_CORES):
        m = {"x": Xb[c * R : (c + 1) * R], "a0": ab}
        if has_bias:
            m["b0"] = np.ascontiguousarray(bias.astype(BF16).reshape(1, D))
        in_maps.append(m)

    res = run_bass_kernel_spmd(
        nc,
        in_maps,
        core_ids=list(range(N_CORES)),
        trace=trace,
        **(trace_kwargs or {}),
    )
    full = np.concatenate(
        [r["out"].astype(np.float32) for r in res.results], axis=0
    )
    return full, res


def kernel(X, alphas, bias):
    try:
        out, _ = _run(X, alphas, bias, trace=False)
    except Exception:
        # One retry for transient device/runtime hiccups.
        out, _ = _run(X, alphas, bias, trace=False)
    return out


# revision 22
# speedup vs baseline: 1.0849x; 1.0849x over previous
"""CrossNet layer kernel for Trainium2 (8 NeuronCores, data parallel).

Computes: out = X * (X @ alphas)[:, None] + bias + X
        = X * (1 + X @ alphas)[:, None] + bias

X: [16384, 4096] f32, alphas: [4096] f32, bias: [4096] f32.

Sharding: X split along batch into 8 row-shards of [2048, 4096]; alphas/bias
replicated (tiny, loaded once per core and broadcast across partitions
on-chip so no replicated DRAM traffic).

The kernel is pure DMA-bound (no data reuse: each X element is read once,
each out element written once), so HBM bytes are the whole cost. The host
quantizes X to bf16 before upload and upcasts the bf16 result after --
device HBM traffic is 32 MiB/core instead of 64 MiB, a 2x win. The row dot
product accumulates in f32, so the only error sources are the bf16
roundings of X / alphas / products / out (~2.7e-3 L2 rel err vs 2e-2 gate).

Measured DMA ceiling: the two HWDGE rings together sustain ~427 GB/s
(SBUF-fabric limit, 16 AXI ports x 32 B x 850 MHz), so the 33.6 MiB
stream needs ~79 us; compute must stay off that critical path.

Per [128, 4104] bf16 tile (x padded with [1,0x7] columns so the reduce
accumulates 1 + x.a directly, folding the "+ X" residual):
  1. DVE tensor_tensor:  o2 = x (*) a      (2x bf16 rate, 2.3 us)
  2. reduce: s1 = sum_free(o2) -- ACT activation(Copy, accum_out) 3.7 us
     for 14 tiles, DVE tensor_reduce 4.4 us for 2 tiles (queue balance:
     both engines ~66 us < 79 us DMA)
  3. DVE tensor_scalar:  o = x * s1        (4x bf16 rate, 1.2 us)
  4. store on the ACT HWDGE ring right behind its producer.
Emission is software-pipelined (reduce 1 tile behind the multiply,
scale/store 2 behind) so cross-engine hops never stall a queue; all 16
x loads are issued upfront into dedicated buffers so the SP ring streams
back-to-back; alphas arrive pre-replicated [128, DP] from the host and
load as the first SP-ring transfer, landing before x0 finishes.
"""

import os
import sys

for _p in ("/opt/trn_rl_repo",):
    if _p not in sys.path and os.path.isdir(_p):
        sys.path.insert(0, _p)

import ml_dtypes
import numpy as np

import concourse.bacc as bacc
import concourse.bass as bass
import concourse.mybir as mybir
from concourse.bass_utils import run_bass_kernel_spmd
from concourse.tile import TileContext

N_CORES = 8
B_FULL = 16384
D = 4096
# X and alphas are padded host-side with 8 extra columns [1,0,0,0,0,0,0,0]:
# the fused multiply-reduce over 4104 elements then accumulates
# 1 + x.a directly (the pad products are 1*1 + 1*0*7), folding the "+ X"
# residual term into the TSP pass with no extra DVE op.
DP = D + 8
R = B_FULL // N_CORES  # rows per core
P = 128  # partitions

BF16 = ml_dtypes.bfloat16

_CACHE = {}


def _build(has_bias: bool) -> bass.Bass:
    f32 = mybir.dt.float32
    bf16 = mybir.dt.bfloat16
    nc = bacc.Bacc("TRN2", target_bir_lowering=False)
    x = nc.dram_tensor("x", (R, DP), bf16, kind="ExternalInput")
    a0 = nc.dram_tensor("a0", (P, DP), bf16, kind="ExternalInput")
    if has_bias:
        b0 = nc.dram_tensor("b0", (P, D), bf16, kind="ExternalInput")
    out = nc.dram_tensor("out", (R, D), bf16, kind="ExternalOutput")

    n_tiles = R // P
    mult = mybir.AluOpType.mult
    add = mybir.AluOpType.add
    bypass = mybir.AluOpType.bypass

    # Per tile: DVE tensor_tensor multiply at the 2x bf16 rate (2.25 us)
    # and DVE tensor_scalar scale at the 4x rate (1.16 us); the free-dim
    # reduce of the products runs on ACT via activation-accum (3.9 us) for
    # most tiles, on DVE tensor_reduce (4.33 us) for DVE_REDUCE tiles to
    # balance the queues (DVE ~63 us, ACT ~65 us total). Emission is
    # software-pipelined (reduce one tile behind the multiply, scale/store
    # two behind) so no engine ever waits on a same-tile cross-engine hop.
    dve_reduce = {0, 8}

    with TileContext(nc) as tc:
        with tc.tile_pool(name="const", bufs=1) as cpool:
            # alphas arrive pre-replicated [128, DP] from the host and load
            # as the FIRST transfer on the SP ring: 1.05 MiB alone at
            # ~427 GB/s lands by ~9.5 us, before x0 finishes. (A GpSimd
            # partition_broadcast stalls >10 us here: its ISA-library load
            # has to queue behind the x-load flood.)
            a_t = cpool.tile([P, DP], bf16)
            nc.sync.dma_start(out=a_t, in_=a0[:, :])
            if has_bias:
                b_t = cpool.tile([P, D], bf16)
                nc.scalar.dma_start(out=b_t, in_=b0[:, :])
            with tc.tile_pool(name="work", bufs=3) as pool:
                # Every x tile gets its own buffer (16 x ~8 KiB/partition):
                # all 16 loads are issued upfront with zero reuse-waits, so
                # the SP ring streams continuously at HBM rate and the
                # compute pipeline never starves.
                x_tiles = []
                for i in range(n_tiles):
                    t = pool.tile([P, DP], bf16, tag="x", bufs=n_tiles)
                    nc.sync.dma_start(out=t, in_=x[i * P : (i + 1) * P, :])
                    x_tiles.append(t)

                o2_tiles = {}
                s1_tiles = {}

                def stage_mul(i):
                    # o2 = x (*) a elementwise, bf16 products (2x DVE rate)
                    o2_t = pool.tile([P, DP], bf16, tag="o2", bufs=3)
                    nc.vector.tensor_tensor(
                        out=o2_t, in0=x_tiles[i], in1=a_t, op=mult
                    )
                    o2_tiles[i] = o2_t

                def stage_reduce(i):
                    # s1 = sum_free(o2) = 1 + x.a (f32 accum; the +1 comes
                    # from the host-side pad columns)
                    s1_t = pool.tile([P, 1], f32, tag="s1", bufs=4)
                    o2_t = o2_tiles[i]
                    if i in dve_reduce:
                        nc.vector.tensor_reduce(
                            out=s1_t,
                            in_=o2_t,
                            axis=mybir.AxisListType.X,
                            op=add,
                        )
                    else:
                        # ACT: dummy in-place copy, accumulator = row sum
                        nc.scalar.activation(
                            out=o2_t,
                            in_=o2_t,
                            func=mybir.ActivationFunctionType.Copy,
                            accum_out=s1_t,
                        )
                    s1_tiles[i] = s1_t

                def stage_scale_store(i):
                    o_t = pool.tile([P, D], bf16, tag="o", bufs=4)
                    if has_bias:
                        nc.vector.scalar_tensor_tensor(
                            out=o_t,
                            in0=x_tiles[i][:, 0:D],
                            scalar=s1_tiles.pop(i),
                            in1=b_t,
                            op0=mult,
                            op1=add,
                        )
                    else:
                        # DVE tensor_scalar: o = x * s1 (4x bf16 rate)
                        nc.vector.tensor_scalar_mul(
                            o_t, x_tiles[i][:, 0:D], s1_tiles.pop(i)
                        )
                    o2_tiles.pop(i)
                    nc.scalar.dma_start(
                        out=out[i * P : (i + 1) * P, :], in_=o_t
                    )

                for i in range(n_tiles):
                    stage_mul(i)
                    if i >= 1:
                        stage_reduce(i - 1)
                    if i >= 2:
                        stage_scale_store(i - 2)
                stage_reduce(n_tiles - 1)
                stage_scale_store(n_tiles - 2)
                stage_scale_store(n_tiles - 1)
    nc.compile()
    return nc


def _run(X, alphas, bias, trace=False, trace_kwargs=None):
    X = np.asarray(X)
    alphas = np.asarray(alphas)
    bias = np.asarray(bias)
    assert X.shape == (B_FULL, D), X.shape

    # Pad columns [1,0,0,0,0,0,0,0] so the on-device multiply-reduce
    # accumulates 1 + x.a directly. alphas are pre-replicated to all 128
    # partitions host-side (1 MiB once) to skip the on-device broadcast.
    pad = np.zeros((1, DP - D), dtype=BF16)
    pad[0, 0] = 1.0
    Xb = np.empty((B_FULL, DP), dtype=BF16)
    Xb[:, :D] = X.astype(BF16)
    Xb[:, D:] = pad
    ab = np.empty((P, DP), dtype=BF16)
    ab[:, :D] = alphas.astype(BF16)[None, :]
    ab[:, D:] = pad

    has_bias = bool(np.any(bias))
    if has_bias not in _CACHE:
        _CACHE[has_bias] = _build(has_bias)
    nc = _CACHE[has_bias]

    in_maps = []
    for c in range(N_CORES):
        m = {"x": Xb[c * R : (c + 1) * R], "a0": ab}
        if has_bias:
            m["b0"] = np.ascontiguousarray(
                np.broadcast_to(bias.astype(BF16)[None, :], (P, D))
            )
        in_maps.append(m)

    res = run_bass_kernel_spmd(
        nc,
        in_maps,
        core_ids=list(range(N_CORES)),
        trace=trace,
        **(trace_kwargs or {}),
    )
    full = np.concatenate(
        [r["out"].astype(np.float32) for r in res.results], axis=0
    )
    return full, res


def kernel(X, alphas, bias):
    try:
        out, _ = _run(X, alphas, bias, trace=False)
    except Exception:
        # One retry for transient device/runtime hiccups.
        out, _ = _run(X, alphas, bias, trace=False)
    return out
